# revision 3
# baseline (speedup 1.0000x reference)
"""Trainium2 Bass kernel for the PriorBCE loss function.

Computes, over full inputs:
  BCE_k   = -mean_b mean_i( log_softmax(L_k, axis=1) * x )   for L_k in {recon_x, logits_rec, logits_text}
  KLD_j   = -0.5 * mean_b mean_e( 1 + lv - mu^2 - exp(lv) )
  W       = mean_b( |mu - pmu|^2 + sum_e( exp(lv) + exp(plv) - 2*exp((lv+plv)/2) ) )
  l       = BCE + 0.5*(KLD1 + KLD2) + W

Strategy: pure data parallel over the batch dim across 8 NeuronCores
(512 rows each). Each core streams its 4 x (512, 20000) fp32 tensors
through SBUF in 128x2500 tiles and produces per-row partial sums:
  dot_k[row]  = sum_i L_k * x     (DVE mul; reduce split DVE / ACT Copy+accum)
  es_k[row]   = sum_i exp(L_k)    (ACT Exp with fused accum_out)
  xs[row]     = sum_i x           (DVE reduce)
plus the tiny (512, 256) KLD/Wasserstein row sums. The host combines
the per-row partials in float64 (log_softmax identity:
  sum_i log_softmax(L)*x = dot - log(es) * xs ).

Engine budget per core (measured-model): DMA 164 MB / ~360 GB/s ~ 455 us;
DVE 5 ops/step * 32 steps * (2500+151)/0.96 ~ 442 us; ACT 5 ops/step *
(2500+352)/1.2 ~ 380 us -> DMA-bound.
"""

import numpy as np

B = 4096
N = 20000
EMB = 256
NCORES = 8
ROWS = B // NCORES  # 512 rows per core
P = 128  # SBUF partitions
RC = ROWS // P  # 4 row chunks per core
F = 2500  # free-dim tile size for the big tensors
CT = N // F  # 8 col tiles per row chunk

_BIG = ("recon_x", "logits_rec", "logits_text")
_ACC_W = RC * CT  # 32 accumulator columns for big-tensor partials

_OUT_BIG = ["dot_0", "dot_1", "dot_2", "es_0", "es_1", "es_2", "xs"]
_OUT_SMALL = ["s_lv", "s_mu2", "s_elv", "s_plv", "s_pmu2", "s_eplv", "s_d2", "s_g"]

_CACHED_NC = None


def _build_nc():
    import concourse.bass as bass  # noqa: F401
    import concourse.tile as tile
    from concourse import bacc, mybir

    fp32 = mybir.dt.float32
    nc = bacc.Bacc("TRN2", target_bir_lowering=False, debug=False, num_devices=NCORES)

    big_in = {k: nc.dram_tensor(k, [ROWS, N], fp32, kind="ExternalInput") for k in _BIG}
    x_in = nc.dram_tensor("x", [ROWS, N], fp32, kind="ExternalInput")
    small_in = {
        k: nc.dram_tensor(k, [ROWS, EMB], fp32, kind="ExternalInput")
        for k in ("mu", "logvar", "prior_mu", "prior_logvar")
    }

    outs = {k: nc.dram_tensor(k, [P, _ACC_W], fp32, kind="ExternalOutput") for k in _OUT_BIG}
    outs.update(
        {k: nc.dram_tensor(k, [P, RC], fp32, kind="ExternalOutput") for k in _OUT_SMALL}
    )

    add = mybir.AluOpType.add
    mult = mybir.AluOpType.mult
    subtract = mybir.AluOpType.subtract
    Exp = mybir.ActivationFunctionType.Exp
    Copy = mybir.ActivationFunctionType.Copy
    AX = mybir.AxisListType.X

    with tile.TileContext(nc) as tc:
        with (
            tc.tile_pool(name="inp", bufs=3) as inp,
            tc.tile_pool(name="scratch", bufs=4) as scratch,
            tc.tile_pool(name="acc", bufs=1) as accp,
            tc.tile_pool(name="smallp", bufs=2) as smallp,
        ):
            acc = {
                k: accp.tile([P, _ACC_W], fp32, tag=f"acc_{k}", name=f"acc_{k}")
                for k in _OUT_BIG
            }
            sacc = {
                k: accp.tile([P, RC], fp32, tag=f"acc_{k}", name=f"acc_{k}")
                for k in _OUT_SMALL
            }
            dummy = accp.tile([P, 1], fp32, name="dummy")

            # ---- big phase: BCE partials ----
            for rc in range(RC):
                r0 = rc * P
                for ct in range(CT):
                    c0 = ct * F
                    col = rc * CT + ct
                    csl = slice(col, col + 1)
                    x_t = inp.tile([P, F], fp32, tag="x_t", name="x_t")
                    nc.sync.dma_start(x_t[:], x_in[r0 : r0 + P, c0 : c0 + F])
                    nc.vector.tensor_reduce(acc["xs"][:, csl], x_t[:], axis=AX, op=add)
                    prods = []
                    for j, nm in enumerate(_BIG):
                        l_t = inp.tile([P, F], fp32, tag=f"l{j}_t", name=f"l{j}_t")
                        nc.sync.dma_start(l_t[:], big_in[nm][r0 : r0 + P, c0 : c0 + F])
                        # sum_i exp(l): ACT, fused accumulate; elementwise out discarded
                        nc.scalar.activation(
                            dummy.broadcast_to(l_t[:].shape),
                            l_t[:],
                            Exp,
                            accum_out=acc[f"es_{j}"][:, csl],
                        )
                        prod = scratch.tile([P, F], fp32, tag="prod", name=f"prod{j}")
                        nc.vector.tensor_tensor(prod[:], l_t[:], x_t[:], op=mult)
                        prods.append(prod)
                    # dot reduces: one on DVE, two on ACT (Copy + accumulate)
                    nc.vector.tensor_reduce(
                        acc["dot_0"][:, csl], prods[0][:], axis=AX, op=add
                    )
                    for j in (1, 2):
                        nc.scalar.activation(
                            dummy.broadcast_to(prods[j][:].shape),
                            prods[j][:],
                            Copy,
                            accum_out=acc[f"dot_{j}"][:, csl],
                        )

            # ---- small phase: KLD / Wasserstein partials ----
            for rc in range(RC):
                r0 = rc * P
                sl = slice(rc, rc + 1)
                mu_t = smallp.tile([P, EMB], fp32, tag="mu_t", name="mu_t")
                lv_t = smallp.tile([P, EMB], fp32, tag="lv_t", name="lv_t")
                pmu_t = smallp.tile([P, EMB], fp32, tag="pmu_t", name="pmu_t")
                plv_t = smallp.tile([P, EMB], fp32, tag="plv_t", name="plv_t")
                nc.sync.dma_start(mu_t[:], small_in["mu"][r0 : r0 + P, :])
                nc.sync.dma_start(lv_t[:], small_in["logvar"][r0 : r0 + P, :])
                nc.sync.dma_start(pmu_t[:], small_in["prior_mu"][r0 : r0 + P, :])
                nc.sync.dma_start(plv_t[:], small_in["prior_logvar"][r0 : r0 + P, :])

                nc.vector.tensor_reduce(sacc["s_lv"][:, sl], lv_t[:], axis=AX, op=add)
                nc.vector.tensor_reduce(sacc["s_plv"][:, sl], plv_t[:], axis=AX, op=add)

                for src, key in ((mu_t, "s_mu2"), (pmu_t, "s_pmu2")):
                    sq = smallp.tile([P, EMB], fp32, tag="sq", name="sq")
                    nc.vector.tensor_tensor(sq[:], src[:], src[:], op=mult)
                    nc.vector.tensor_reduce(sacc[key][:, sl], sq[:], axis=AX, op=add)

                d_t = smallp.tile([P, EMB], fp32, tag="d_t", name="d_t")
                nc.vector.tensor_tensor(d_t[:], mu_t[:], pmu_t[:], op=subtract)
                sqd = smallp.tile([P, EMB], fp32, tag="sq", name="sqd")
                nc.vector.tensor_tensor(sqd[:], d_t[:], d_t[:], op=mult)
                nc.vector.tensor_reduce(sacc["s_d2"][:, sl], sqd[:], axis=AX, op=add)

                sum_t = smallp.tile([P, EMB], fp32, tag="sum_t", name="sum_t")
                nc.vector.tensor_tensor(sum_t[:], lv_t[:], plv_t[:], op=add)

                nc.scalar.activation(
                    dummy.broadcast_to(lv_t[:].shape), lv_t[:], Exp,
                    accum_out=sacc["s_elv"][:, sl],
                )
                nc.scalar.activation(
                    dummy.broadcast_to(plv_t[:].shape), plv_t[:], Exp,
                    accum_out=sacc["s_eplv"][:, sl],
                )
                nc.scalar.activation(
                    dummy.broadcast_to(sum_t[:].shape), sum_t[:], Exp, scale=0.5,
                    accum_out=sacc["s_g"][:, sl],
                )

            # ---- write partials out ----
            for k in _OUT_BIG:
                nc.sync.dma_start(outs[k][:, :], acc[k][:])
            for k in _OUT_SMALL:
                nc.sync.dma_start(outs[k][:, :], sacc[k][:])

    nc.compile()
    return nc


def _get_nc():
    global _CACHED_NC
    if _CACHED_NC is None:
        _CACHED_NC = _build_nc()
    return _CACHED_NC


LAST_RESULTS = None


def _combine(results):
    """Combine per-core per-row partial sums into the six scalars (float64)."""
    tot_bce = np.zeros(3, dtype=np.float64)
    tot_kld1 = 0.0
    tot_kld2 = 0.0
    tot_w = 0.0
    for r in results:
        xs = r["xs"].astype(np.float64).reshape(P, RC, CT).sum(-1)
        for j in range(3):
            dot = r[f"dot_{j}"].astype(np.float64).reshape(P, RC, CT).sum(-1)
            es = r[f"es_{j}"].astype(np.float64).reshape(P, RC, CT).sum(-1)
            tot_bce[j] += (dot - np.log(es) * xs).sum()
        s_lv = r["s_lv"].astype(np.float64)
        s_mu2 = r["s_mu2"].astype(np.float64)
        s_elv = r["s_elv"].astype(np.float64)
        s_plv = r["s_plv"].astype(np.float64)
        s_pmu2 = r["s_pmu2"].astype(np.float64)
        s_eplv = r["s_eplv"].astype(np.float64)
        s_d2 = r["s_d2"].astype(np.float64)
        s_g = r["s_g"].astype(np.float64)
        tot_kld1 += (EMB + s_lv - s_mu2 - s_elv).sum()
        tot_kld2 += (EMB + s_plv - s_pmu2 - s_eplv).sum()
        tot_w += (s_d2 + s_elv + s_eplv - 2.0 * s_g).sum()

    BCE_merged = -tot_bce[0] / (B * N)  # recon_x
    BCE_rec = -tot_bce[1] / (B * N)  # logits_rec
    BCE_text = -tot_bce[2] / (B * N)  # logits_text
    BCE = (BCE_merged + BCE_text + BCE_rec) / 3.0
    KLD1 = -0.5 * tot_kld1 / (B * EMB)
    KLD2 = -0.5 * tot_kld2 / (B * EMB)
    W = tot_w / B
    l = BCE + 0.5 * (KLD1 + KLD2) + W
    return tuple(np.float32(v) for v in (l, BCE, W, BCE_rec, BCE_text, BCE_merged))


def kernel(recon_x, logits_rec, logits_text, x, mu, logvar, prior_mu, prior_logvar):
    from concourse.bass_utils import run_bass_kernel_spmd

    global LAST_RESULTS
    full = {
        "recon_x": recon_x,
        "logits_rec": logits_rec,
        "logits_text": logits_text,
        "x": x,
        "mu": mu,
        "logvar": logvar,
        "prior_mu": prior_mu,
        "prior_logvar": prior_logvar,
    }
    full = {k: np.ascontiguousarray(np.asarray(v, dtype=np.float32)) for k, v in full.items()}

    in_maps = [
        {k: v[i * ROWS : (i + 1) * ROWS] for k, v in full.items()} for i in range(NCORES)
    ]
    nc = _get_nc()
    LAST_RESULTS = run_bass_kernel_spmd(nc, in_maps, list(range(NCORES)))
    return _combine(LAST_RESULTS.results)


# revision 6
# speedup vs baseline: 182.5752x; 182.5752x over previous
"""Trainium2 Bass kernel for the PriorBCE loss function.

Computes, over full inputs:
  BCE_k   = -mean_b mean_i( log_softmax(L_k, axis=1) * x )   for L_k in {recon_x, logits_rec, logits_text}
  KLD_j   = -0.5 * mean_b mean_e( 1 + lv - mu^2 - exp(lv) )
  W       = mean_b( |mu - pmu|^2 + sum_e( exp(lv) + exp(plv) - 2*exp((lv+plv)/2) ) )
  l       = BCE + 0.5*(KLD1 + KLD2) + W

Strategy: pure data parallel over the batch dim across 8 NeuronCores
(512 rows each). Each core streams its 4 x (512, 20000) fp32 tensors
through SBUF in 128x2500 tiles and produces per-row partial sums:
  dot_k[row]  = sum_i L_k * x     (DVE mul; reduce split DVE / ACT Copy+accum)
  es_k[row]   = sum_i exp(L_k)    (ACT Exp with fused accum_out)
  xs[row]     = sum_i x           (DVE reduce)
plus the tiny (512, 256) KLD/Wasserstein row sums. The host combines
the per-row partials in float64 (log_softmax identity:
  sum_i log_softmax(L)*x = dot - log(es) * xs ).

Engine budget per core (measured-model): DMA 164 MB / ~360 GB/s ~ 455 us;
DVE 5 ops/step * 32 steps * (2500+151)/0.96 ~ 442 us; ACT 5 ops/step *
(2500+352)/1.2 ~ 380 us -> DMA-bound.
"""

import numpy as np

B = 4096
N = 20000
EMB = 256
NCORES = 8
ROWS = B // NCORES  # 512 rows per core
P = 128  # SBUF partitions
RC = ROWS // P  # 4 row chunks per core
F = 2500  # free-dim tile size for the big tensors
CT = N // F  # 8 col tiles per row chunk

_BIG = ("recon_x", "logits_rec", "logits_text")
_ACC_W = RC * CT  # 32 accumulator columns for big-tensor partials

_OUT_BIG = ["dot_0", "dot_1", "dot_2", "es_0", "es_1", "es_2", "xs"]
_OUT_SMALL = ["s_lv", "s_mu2", "s_elv", "s_plv", "s_pmu2", "s_eplv", "s_d2", "s_g"]

_CACHED_NC = None


def _build_nc(n_repeat=1):
    """n_repeat > 1 re-emits the whole compute body (same inputs, same
    accumulators) for slope-based timing; results are unchanged."""
    import concourse.bass as bass  # noqa: F401
    import concourse.tile as tile
    from concourse import bacc, mybir

    fp32 = mybir.dt.float32
    nc = bacc.Bacc("TRN2", target_bir_lowering=False, debug=False, num_devices=NCORES)

    big_in = {k: nc.dram_tensor(k, [ROWS, N], fp32, kind="ExternalInput") for k in _BIG}
    x_in = nc.dram_tensor("x", [ROWS, N], fp32, kind="ExternalInput")
    small_in = {
        k: nc.dram_tensor(k, [ROWS, EMB], fp32, kind="ExternalInput")
        for k in ("mu", "logvar", "prior_mu", "prior_logvar")
    }

    outs = {k: nc.dram_tensor(k, [P, _ACC_W], fp32, kind="ExternalOutput") for k in _OUT_BIG}
    outs.update(
        {k: nc.dram_tensor(k, [P, RC], fp32, kind="ExternalOutput") for k in _OUT_SMALL}
    )

    add = mybir.AluOpType.add
    mult = mybir.AluOpType.mult
    subtract = mybir.AluOpType.subtract
    Exp = mybir.ActivationFunctionType.Exp
    Copy = mybir.ActivationFunctionType.Copy
    AX = mybir.AxisListType.X

    with tile.TileContext(nc) as tc:
        with (
            tc.tile_pool(name="inp", bufs=3) as inp,
            tc.tile_pool(name="scratch", bufs=4) as scratch,
            tc.tile_pool(name="acc", bufs=1) as accp,
            tc.tile_pool(name="smallp", bufs=2) as smallp,
        ):
            acc = {
                k: accp.tile([P, _ACC_W], fp32, tag=f"acc_{k}", name=f"acc_{k}")
                for k in _OUT_BIG
            }
            sacc = {
                k: accp.tile([P, RC], fp32, tag=f"acc_{k}", name=f"acc_{k}")
                for k in _OUT_SMALL
            }
            dummy = accp.tile([P, 1], fp32, name="dummy")

            def _emit_big():
                for rc in range(RC):
                    r0 = rc * P
                    for ct in range(CT):
                        c0 = ct * F
                        col = rc * CT + ct
                        csl = slice(col, col + 1)
                        x_t = inp.tile([P, F], fp32, tag="x_t", name="x_t")
                        nc.sync.dma_start(x_t[:], x_in[r0 : r0 + P, c0 : c0 + F])
                        nc.vector.tensor_reduce(
                            acc["xs"][:, csl], x_t[:], axis=AX, op=add
                        )
                        prods = []
                        for j, nm in enumerate(_BIG):
                            l_t = inp.tile([P, F], fp32, tag=f"l{j}_t", name=f"l{j}_t")
                            nc.sync.dma_start(
                                l_t[:], big_in[nm][r0 : r0 + P, c0 : c0 + F]
                            )
                            # sum_i exp(l): ACT, fused accumulate; elementwise out discarded
                            nc.scalar.activation(
                                dummy.broadcast_to(l_t[:].shape),
                                l_t[:],
                                Exp,
                                accum_out=acc[f"es_{j}"][:, csl],
                            )
                            prod = scratch.tile([P, F], fp32, tag="prod", name=f"prod{j}")
                            nc.vector.tensor_tensor(prod[:], l_t[:], x_t[:], op=mult)
                            prods.append(prod)
                        # dot reduces: one on DVE, two on ACT (Copy + accumulate)
                        nc.vector.tensor_reduce(
                            acc["dot_0"][:, csl], prods[0][:], axis=AX, op=add
                        )
                        for j in (1, 2):
                            nc.scalar.activation(
                                dummy.broadcast_to(prods[j][:].shape),
                                prods[j][:],
                                Copy,
                                accum_out=acc[f"dot_{j}"][:, csl],
                            )

            def _emit_small():
                for rc in range(RC):
                    r0 = rc * P
                    sl = slice(rc, rc + 1)
                    mu_t = smallp.tile([P, EMB], fp32, tag="mu_t", name="mu_t")
                    lv_t = smallp.tile([P, EMB], fp32, tag="lv_t", name="lv_t")
                    pmu_t = smallp.tile([P, EMB], fp32, tag="pmu_t", name="pmu_t")
                    plv_t = smallp.tile([P, EMB], fp32, tag="plv_t", name="plv_t")
                    nc.sync.dma_start(mu_t[:], small_in["mu"][r0 : r0 + P, :])
                    nc.sync.dma_start(lv_t[:], small_in["logvar"][r0 : r0 + P, :])
                    nc.sync.dma_start(pmu_t[:], small_in["prior_mu"][r0 : r0 + P, :])
                    nc.sync.dma_start(plv_t[:], small_in["prior_logvar"][r0 : r0 + P, :])

                    nc.vector.tensor_reduce(sacc["s_lv"][:, sl], lv_t[:], axis=AX, op=add)
                    nc.vector.tensor_reduce(
                        sacc["s_plv"][:, sl], plv_t[:], axis=AX, op=add
                    )

                    for src, key in ((mu_t, "s_mu2"), (pmu_t, "s_pmu2")):
                        sq = smallp.tile([P, EMB], fp32, tag="sq", name="sq")
                        nc.vector.tensor_tensor(sq[:], src[:], src[:], op=mult)
                        nc.vector.tensor_reduce(sacc[key][:, sl], sq[:], axis=AX, op=add)

                    d_t = smallp.tile([P, EMB], fp32, tag="d_t", name="d_t")
                    nc.vector.tensor_tensor(d_t[:], mu_t[:], pmu_t[:], op=subtract)
                    sqd = smallp.tile([P, EMB], fp32, tag="sq", name="sqd")
                    nc.vector.tensor_tensor(sqd[:], d_t[:], d_t[:], op=mult)
                    nc.vector.tensor_reduce(sacc["s_d2"][:, sl], sqd[:], axis=AX, op=add)

                    sum_t = smallp.tile([P, EMB], fp32, tag="sum_t", name="sum_t")
                    nc.vector.tensor_tensor(sum_t[:], lv_t[:], plv_t[:], op=add)

                    nc.scalar.activation(
                        dummy.broadcast_to(lv_t[:].shape), lv_t[:], Exp,
                        accum_out=sacc["s_elv"][:, sl],
                    )
                    nc.scalar.activation(
                        dummy.broadcast_to(plv_t[:].shape), plv_t[:], Exp,
                        accum_out=sacc["s_eplv"][:, sl],
                    )
                    nc.scalar.activation(
                        dummy.broadcast_to(sum_t[:].shape), sum_t[:], Exp, scale=0.5,
                        accum_out=sacc["s_g"][:, sl],
                    )

            for _rep in range(n_repeat):
                _emit_big()
                _emit_small()

            # ---- write partials out ----
            for k in _OUT_BIG:
                nc.sync.dma_start(outs[k][:, :], acc[k][:])
            for k in _OUT_SMALL:
                nc.sync.dma_start(outs[k][:, :], sacc[k][:])

    nc.compile()
    return nc


def _get_nc():
    global _CACHED_NC
    if _CACHED_NC is None:
        _CACHED_NC = _build_nc()
    return _CACHED_NC


LAST_RESULTS = None


def _combine(results):
    """Combine per-core per-row partial sums into the six scalars (float64)."""
    tot_bce = np.zeros(3, dtype=np.float64)
    tot_kld1 = 0.0
    tot_kld2 = 0.0
    tot_w = 0.0
    for r in results:
        xs = r["xs"].astype(np.float64).reshape(P, RC, CT).sum(-1)
        for j in range(3):
            dot = r[f"dot_{j}"].astype(np.float64).reshape(P, RC, CT).sum(-1)
            es = r[f"es_{j}"].astype(np.float64).reshape(P, RC, CT).sum(-1)
            tot_bce[j] += (dot - np.log(es) * xs).sum()
        s_lv = r["s_lv"].astype(np.float64)
        s_mu2 = r["s_mu2"].astype(np.float64)
        s_elv = r["s_elv"].astype(np.float64)
        s_plv = r["s_plv"].astype(np.float64)
        s_pmu2 = r["s_pmu2"].astype(np.float64)
        s_eplv = r["s_eplv"].astype(np.float64)
        s_d2 = r["s_d2"].astype(np.float64)
        s_g = r["s_g"].astype(np.float64)
        tot_kld1 += (EMB + s_lv - s_mu2 - s_elv).sum()
        tot_kld2 += (EMB + s_plv - s_pmu2 - s_eplv).sum()
        tot_w += (s_d2 + s_elv + s_eplv - 2.0 * s_g).sum()

    BCE_merged = -tot_bce[0] / (B * N)  # recon_x
    BCE_rec = -tot_bce[1] / (B * N)  # logits_rec
    BCE_text = -tot_bce[2] / (B * N)  # logits_text
    BCE = (BCE_merged + BCE_text + BCE_rec) / 3.0
    KLD1 = -0.5 * tot_kld1 / (B * EMB)
    KLD2 = -0.5 * tot_kld2 / (B * EMB)
    W = tot_w / B
    l = BCE + 0.5 * (KLD1 + KLD2) + W
    return tuple(np.float32(v) for v in (l, BCE, W, BCE_rec, BCE_text, BCE_merged))


def kernel(recon_x, logits_rec, logits_text, x, mu, logvar, prior_mu, prior_logvar):
    from concourse.bass_utils import run_bass_kernel_spmd

    global LAST_RESULTS
    full = {
        "recon_x": recon_x,
        "logits_rec": logits_rec,
        "logits_text": logits_text,
        "x": x,
        "mu": mu,
        "logvar": logvar,
        "prior_mu": prior_mu,
        "prior_logvar": prior_logvar,
    }
    full = {k: np.ascontiguousarray(np.asarray(v, dtype=np.float32)) for k, v in full.items()}

    in_maps = [
        {k: v[i * ROWS : (i + 1) * ROWS] for k, v in full.items()} for i in range(NCORES)
    ]
    nc = _get_nc()
    LAST_RESULTS = run_bass_kernel_spmd(nc, in_maps, list(range(NCORES)))
    return _combine(LAST_RESULTS.results)


# revision 11
# speedup vs baseline: 216.5843x; 1.1863x over previous
"""Trainium2 Bass kernel for the PriorBCE loss function.

Computes, over full inputs:
  BCE_k   = -mean_b mean_i( log_softmax(L_k, axis=1) * x )   for L_k in {recon_x, logits_rec, logits_text}
  KLD_j   = -0.5 * mean_b mean_e( 1 + lv - mu^2 - exp(lv) )
  W       = mean_b( |mu - pmu|^2 + sum_e( exp(lv) + exp(plv) - 2*exp((lv+plv)/2) ) )
  l       = BCE + 0.5*(KLD1 + KLD2) + W

Strategy: pure data parallel over the batch dim across 8 NeuronCores
(512 rows each). Each core streams its 4 x (512, 20000) fp32 tensors
through SBUF in 128x2500 tiles and produces per-row partial sums:
  dot_k[row]  = sum_i L_k * x     (DVE mul; reduce split DVE / ACT Copy+accum)
  es_k[row]   = sum_i exp(L_k)    (ACT Exp with fused accum_out)
  xs[row]     = sum_i x           (DVE reduce)
plus the tiny (512, 256) KLD/Wasserstein row sums. The host combines
the per-row partials in float64 (log_softmax identity:
  sum_i log_softmax(L)*x = dot - log(es) * xs ).

Engine budget per core (measured-model): DMA 164 MB / ~360 GB/s ~ 455 us;
DVE 5 ops/step * 32 steps * (2500+151)/0.96 ~ 442 us; ACT 5 ops/step *
(2500+352)/1.2 ~ 380 us -> DMA-bound.
"""

import numpy as np

B = 4096
N = 20000
EMB = 256
NCORES = 8
ROWS = B // NCORES  # 512 rows per core
P = 128  # SBUF partitions
RC = ROWS // P  # 4 row chunks per core
F = 2500  # free-dim tile size for the big tensors
CT = N // F  # 8 col tiles per row chunk

_BIG = ("recon_x", "logits_rec", "logits_text")
_ACC_W = RC * CT  # 32 accumulator columns for big-tensor partials

_OUT_BIG = ["dot_0", "dot_1", "dot_2", "es_0", "es_1", "es_2", "xs"]
_OUT_SMALL = ["s_lv", "s_mu2", "s_elv", "s_plv", "s_pmu2", "s_eplv", "s_d2", "s_g"]

_CACHED_NC = None


# Issuing some loads on the scalar HWDGE ring measured ~230 us slower
# (DMA issue serializes against ACT compute in the ACT stream) — keep all
# loads on the sync ring.
SPLIT_DMA = False


def _build_nc(n_repeat=1, split_dma=None):
    """n_repeat > 1 re-emits the whole compute body (same inputs, same
    accumulators) for slope-based timing; results are unchanged."""
    if split_dma is None:
        split_dma = SPLIT_DMA
    import concourse.bass as bass  # noqa: F401
    import concourse.tile as tile
    from concourse import bacc, mybir

    fp32 = mybir.dt.float32
    nc = bacc.Bacc("TRN2", target_bir_lowering=False, debug=False, num_devices=NCORES)

    big_in = {k: nc.dram_tensor(k, [ROWS, N], fp32, kind="ExternalInput") for k in _BIG}
    x_in = nc.dram_tensor("x", [ROWS, N], fp32, kind="ExternalInput")
    small_in = {
        k: nc.dram_tensor(k, [ROWS, EMB], fp32, kind="ExternalInput")
        for k in ("mu", "logvar", "prior_mu", "prior_logvar")
    }

    outs = {k: nc.dram_tensor(k, [P, _ACC_W], fp32, kind="ExternalOutput") for k in _OUT_BIG}
    outs.update(
        {k: nc.dram_tensor(k, [P, RC], fp32, kind="ExternalOutput") for k in _OUT_SMALL}
    )

    add = mybir.AluOpType.add
    mult = mybir.AluOpType.mult
    subtract = mybir.AluOpType.subtract
    Exp = mybir.ActivationFunctionType.Exp
    Copy = mybir.ActivationFunctionType.Copy
    AX = mybir.AxisListType.X

    with tile.TileContext(nc) as tc:
        with (
            tc.tile_pool(name="inp", bufs=3) as inp,
            tc.tile_pool(name="scratch", bufs=4) as scratch,
            tc.tile_pool(name="acc", bufs=1) as accp,
            tc.tile_pool(name="smallp", bufs=1) as smallp,
        ):
            acc = {
                k: accp.tile([P, _ACC_W], fp32, tag=f"acc_{k}", name=f"acc_{k}")
                for k in _OUT_BIG
            }
            sacc = {
                k: accp.tile([P, RC], fp32, tag=f"acc_{k}", name=f"acc_{k}")
                for k in _OUT_SMALL
            }
            dummy = accp.tile([P, 1], fp32, name="dummy")

            def _emit_big():
                for rc in range(RC):
                    r0 = rc * P
                    for ct in range(CT):
                        c0 = ct * F
                        col = rc * CT + ct
                        csl = slice(col, col + 1)
                        x_t = inp.tile([P, F], fp32, tag="x_t", name="x_t")
                        nc.sync.dma_start(x_t[:], x_in[r0 : r0 + P, c0 : c0 + F])
                        nc.vector.tensor_reduce(
                            acc["xs"][:, csl], x_t[:], axis=AX, op=add
                        )
                        prods = []
                        for j, nm in enumerate(_BIG):
                            l_t = inp.tile([P, F], fp32, tag=f"l{j}_t", name=f"l{j}_t")
                            eng = nc.scalar if (split_dma and j >= 1) else nc.sync
                            eng.dma_start(
                                l_t[:], big_in[nm][r0 : r0 + P, c0 : c0 + F]
                            )
                            # sum_i exp(l): ACT, fused accumulate; elementwise out discarded
                            nc.scalar.activation(
                                dummy.broadcast_to(l_t[:].shape),
                                l_t[:],
                                Exp,
                                accum_out=acc[f"es_{j}"][:, csl],
                            )
                            prod = scratch.tile([P, F], fp32, tag="prod", name=f"prod{j}")
                            nc.vector.tensor_tensor(prod[:], l_t[:], x_t[:], op=mult)
                            prods.append(prod)
                        # dot reduces: one on DVE, two on ACT (Copy + accumulate)
                        nc.vector.tensor_reduce(
                            acc["dot_0"][:, csl], prods[0][:], axis=AX, op=add
                        )
                        for j in (1, 2):
                            nc.scalar.activation(
                                dummy.broadcast_to(prods[j][:].shape),
                                prods[j][:],
                                Copy,
                                accum_out=acc[f"dot_{j}"][:, csl],
                            )

            def _emit_small():
                # Batched: all RC row-chunks in one [P, RC*EMB] tile per tensor.
                W = RC * EMB
                tiles = {}
                for k in ("mu", "logvar", "prior_mu", "prior_logvar"):
                    t = smallp.tile([P, W], fp32, tag=f"sm_{k}", name=f"sm_{k}")
                    src = small_in[k][:, :].rearrange("(c p) e -> p c e", p=P)
                    nc.sync.dma_start(t[:].rearrange("p (c e) -> p c e", c=RC), src)
                    tiles[k] = t

                mu_t, lv_t = tiles["mu"], tiles["logvar"]
                pmu_t, plv_t = tiles["prior_mu"], tiles["prior_logvar"]

                def red(dst, src_t):
                    nc.vector.tensor_reduce(
                        dst[:, :], src_t[:].rearrange("p (c e) -> p c e", c=RC),
                        axis=AX, op=add,
                    )

                red(sacc["s_lv"], lv_t)
                red(sacc["s_plv"], plv_t)
                for src, key in ((mu_t, "s_mu2"), (pmu_t, "s_pmu2")):
                    sq = smallp.tile([P, W], fp32, tag="sq", name="sq")
                    nc.vector.tensor_tensor(sq[:], src[:], src[:], op=mult)
                    red(sacc[key], sq)
                d_t = smallp.tile([P, W], fp32, tag="d_t", name="d_t")
                nc.vector.tensor_tensor(d_t[:], mu_t[:], pmu_t[:], op=subtract)
                sqd = smallp.tile([P, W], fp32, tag="sq", name="sqd")
                nc.vector.tensor_tensor(sqd[:], d_t[:], d_t[:], op=mult)
                red(sacc["s_d2"], sqd)
                sum_t = smallp.tile([P, W], fp32, tag="sum_t", name="sum_t")
                nc.vector.tensor_tensor(sum_t[:], lv_t[:], plv_t[:], op=add)

                # ACT accum_out must be one scalar per partition -> per-chunk ops
                for rc in range(RC):
                    sl = slice(rc, rc + 1)
                    esl = slice(rc * EMB, (rc + 1) * EMB)
                    nc.scalar.activation(
                        dummy.broadcast_to(lv_t[:, esl].shape), lv_t[:, esl], Exp,
                        accum_out=sacc["s_elv"][:, sl],
                    )
                    nc.scalar.activation(
                        dummy.broadcast_to(plv_t[:, esl].shape), plv_t[:, esl], Exp,
                        accum_out=sacc["s_eplv"][:, sl],
                    )
                    nc.scalar.activation(
                        dummy.broadcast_to(sum_t[:, esl].shape), sum_t[:, esl], Exp,
                        scale=0.5, accum_out=sacc["s_g"][:, sl],
                    )

            for _rep in range(n_repeat):
                _emit_small()
                _emit_big()

            # ---- write partials out ----
            for k in _OUT_BIG:
                nc.sync.dma_start(outs[k][:, :], acc[k][:])
            for k in _OUT_SMALL:
                nc.sync.dma_start(outs[k][:, :], sacc[k][:])

    nc.compile()
    return nc


def _get_nc():
    global _CACHED_NC
    if _CACHED_NC is None:
        _CACHED_NC = _build_nc()
    return _CACHED_NC


LAST_RESULTS = None


def _combine(results):
    """Combine per-core per-row partial sums into the six scalars (float64)."""
    tot_bce = np.zeros(3, dtype=np.float64)
    tot_kld1 = 0.0
    tot_kld2 = 0.0
    tot_w = 0.0
    for r in results:
        xs = r["xs"].astype(np.float64).reshape(P, RC, CT).sum(-1)
        for j in range(3):
            dot = r[f"dot_{j}"].astype(np.float64).reshape(P, RC, CT).sum(-1)
            es = r[f"es_{j}"].astype(np.float64).reshape(P, RC, CT).sum(-1)
            tot_bce[j] += (dot - np.log(es) * xs).sum()
        s_lv = r["s_lv"].astype(np.float64)
        s_mu2 = r["s_mu2"].astype(np.float64)
        s_elv = r["s_elv"].astype(np.float64)
        s_plv = r["s_plv"].astype(np.float64)
        s_pmu2 = r["s_pmu2"].astype(np.float64)
        s_eplv = r["s_eplv"].astype(np.float64)
        s_d2 = r["s_d2"].astype(np.float64)
        s_g = r["s_g"].astype(np.float64)
        tot_kld1 += (EMB + s_lv - s_mu2 - s_elv).sum()
        tot_kld2 += (EMB + s_plv - s_pmu2 - s_eplv).sum()
        tot_w += (s_d2 + s_elv + s_eplv - 2.0 * s_g).sum()

    BCE_merged = -tot_bce[0] / (B * N)  # recon_x
    BCE_rec = -tot_bce[1] / (B * N)  # logits_rec
    BCE_text = -tot_bce[2] / (B * N)  # logits_text
    BCE = (BCE_merged + BCE_text + BCE_rec) / 3.0
    KLD1 = -0.5 * tot_kld1 / (B * EMB)
    KLD2 = -0.5 * tot_kld2 / (B * EMB)
    W = tot_w / B
    l = BCE + 0.5 * (KLD1 + KLD2) + W
    return tuple(np.float32(v) for v in (l, BCE, W, BCE_rec, BCE_text, BCE_merged))


def kernel(recon_x, logits_rec, logits_text, x, mu, logvar, prior_mu, prior_logvar):
    from concourse.bass_utils import run_bass_kernel_spmd

    global LAST_RESULTS
    full = {
        "recon_x": recon_x,
        "logits_rec": logits_rec,
        "logits_text": logits_text,
        "x": x,
        "mu": mu,
        "logvar": logvar,
        "prior_mu": prior_mu,
        "prior_logvar": prior_logvar,
    }
    full = {k: np.ascontiguousarray(np.asarray(v, dtype=np.float32)) for k, v in full.items()}

    in_maps = [
        {k: v[i * ROWS : (i + 1) * ROWS] for k, v in full.items()} for i in range(NCORES)
    ]
    nc = _get_nc()
    LAST_RESULTS = run_bass_kernel_spmd(nc, in_maps, list(range(NCORES)))
    return _combine(LAST_RESULTS.results)


# revision 17
# speedup vs baseline: 219.2330x; 1.0122x over previous
"""Trainium2 Bass kernel for the PriorBCE loss function.

Computes, over full inputs:
  BCE_k   = -mean_b mean_i( log_softmax(L_k, axis=1) * x )   for L_k in {recon_x, logits_rec, logits_text}
  KLD_j   = -0.5 * mean_b mean_e( 1 + lv - mu^2 - exp(lv) )
  W       = mean_b( |mu - pmu|^2 + sum_e( exp(lv) + exp(plv) - 2*exp((lv+plv)/2) ) )
  l       = BCE + 0.5*(KLD1 + KLD2) + W

Strategy: pure data parallel over the batch dim across 8 NeuronCores
(512 rows each). Each core streams its 4 x (512, 20000) fp32 tensors
through SBUF in 128x2500 tiles and produces per-row partial sums:
  dot_k[row]  = sum_i L_k * x     (DVE mul; reduce split DVE / ACT Copy+accum)
  es_k[row]   = sum_i exp(L_k)    (ACT Exp with fused accum_out)
  xs[row]     = sum_i x           (DVE reduce)
plus the tiny (512, 256) KLD/Wasserstein row sums. The host combines
the per-row partials in float64 (log_softmax identity:
  sum_i log_softmax(L)*x = dot - log(es) * xs ).

Engine budget per core (measured-model): DMA 164 MB / ~360 GB/s ~ 455 us;
DVE 5 ops/step * 32 steps * (2500+151)/0.96 ~ 442 us; ACT 5 ops/step *
(2500+352)/1.2 ~ 380 us -> DMA-bound.
"""

import numpy as np

B = 4096
N = 20000
EMB = 256
NCORES = 8
ROWS = B // NCORES  # 512 rows per core
P = 128  # SBUF partitions
RC = ROWS // P  # 4 row chunks per core
F = 2500  # free-dim tile size for the big tensors
CT = N // F  # 8 col tiles per row chunk

_BIG = ("recon_x", "logits_rec", "logits_text")
_ACC_W = RC * CT  # 32 accumulator columns for big-tensor partials

_OUT_BIG = ["dot_0", "dot_1", "dot_2", "es_0", "es_1", "es_2", "xs"]
_OUT_SMALL = ["s_lv", "s_mu2", "s_elv", "s_plv", "s_pmu2", "s_eplv", "s_d2", "s_g"]

_CACHED_NC = None


# Issuing some loads on the scalar HWDGE ring measured ~230 us slower
# (DMA issue serializes against ACT compute in the ACT stream) — keep all
# loads on the sync ring.
SPLIT_DMA = False


# Alternating the dot_0 reduce DVE/ACT (4.5/5.5 ops per step) measured
# ~3.5 us/body slower in an in-process A/B at R36 — the kernel is DMA-bound,
# not engine-bound. Keep the straight 5/5 split.
BALANCE = False


def _build_nc(n_repeat=1, split_dma=None, balance=None):
    """n_repeat > 1 re-emits the whole compute body (same inputs, same
    accumulators) for slope-based timing; results are unchanged."""
    if split_dma is None:
        split_dma = SPLIT_DMA
    if balance is None:
        balance = BALANCE
    import concourse.bass as bass  # noqa: F401
    import concourse.tile as tile
    from concourse import bacc, mybir

    fp32 = mybir.dt.float32
    nc = bacc.Bacc("TRN2", target_bir_lowering=False, debug=False, num_devices=NCORES)

    big_in = {k: nc.dram_tensor(k, [ROWS, N], fp32, kind="ExternalInput") for k in _BIG}
    x_in = nc.dram_tensor("x", [ROWS, N], fp32, kind="ExternalInput")
    small_in = {
        k: nc.dram_tensor(k, [ROWS, EMB], fp32, kind="ExternalInput")
        for k in ("mu", "logvar", "prior_mu", "prior_logvar")
    }

    outs = {k: nc.dram_tensor(k, [P, _ACC_W], fp32, kind="ExternalOutput") for k in _OUT_BIG}
    outs.update(
        {k: nc.dram_tensor(k, [P, RC], fp32, kind="ExternalOutput") for k in _OUT_SMALL}
    )

    add = mybir.AluOpType.add
    mult = mybir.AluOpType.mult
    subtract = mybir.AluOpType.subtract
    Exp = mybir.ActivationFunctionType.Exp
    Copy = mybir.ActivationFunctionType.Copy
    AX = mybir.AxisListType.X

    with tile.TileContext(nc) as tc:
        with (
            tc.tile_pool(name="inp", bufs=3) as inp,
            tc.tile_pool(name="scratch", bufs=4) as scratch,
            tc.tile_pool(name="acc", bufs=1) as accp,
            tc.tile_pool(name="smallp", bufs=1) as smallp,
        ):
            acc = {
                k: accp.tile([P, _ACC_W], fp32, tag=f"acc_{k}", name=f"acc_{k}")
                for k in _OUT_BIG
            }
            sacc = {
                k: accp.tile([P, RC], fp32, tag=f"acc_{k}", name=f"acc_{k}")
                for k in _OUT_SMALL
            }
            dummy = accp.tile([P, 1], fp32, name="dummy")

            def _emit_big_rc(rc):
                    r0 = rc * P
                    for ct in range(CT):
                        c0 = ct * F
                        col = rc * CT + ct
                        csl = slice(col, col + 1)
                        x_t = inp.tile([P, F], fp32, tag="x_t", name="x_t")
                        nc.sync.dma_start(x_t[:], x_in[r0 : r0 + P, c0 : c0 + F])
                        nc.vector.tensor_reduce(
                            acc["xs"][:, csl], x_t[:], axis=AX, op=add
                        )
                        prods = []
                        for j, nm in enumerate(_BIG):
                            l_t = inp.tile([P, F], fp32, tag=f"l{j}_t", name=f"l{j}_t")
                            eng = nc.scalar if (split_dma and j >= 1) else nc.sync
                            eng.dma_start(
                                l_t[:], big_in[nm][r0 : r0 + P, c0 : c0 + F]
                            )
                            # sum_i exp(l): ACT, fused accumulate; elementwise out discarded
                            nc.scalar.activation(
                                dummy.broadcast_to(l_t[:].shape),
                                l_t[:],
                                Exp,
                                accum_out=acc[f"es_{j}"][:, csl],
                            )
                            prod = scratch.tile([P, F], fp32, tag="prod", name=f"prod{j}")
                            nc.vector.tensor_tensor(prod[:], l_t[:], x_t[:], op=mult)
                            prods.append(prod)
                        # dot reduces: split DVE/ACT; with balance, dot_0
                        # goes to ACT on odd steps (DVE avg 4.5 ops/step,
                        # ACT 5.5 — both under the DMA floor)
                        dve_dot0 = not (balance and (rc * CT + ct) % 2 == 1)
                        if dve_dot0:
                            nc.vector.tensor_reduce(
                                acc["dot_0"][:, csl], prods[0][:], axis=AX, op=add
                            )
                        else:
                            nc.scalar.activation(
                                dummy.broadcast_to(prods[0][:].shape),
                                prods[0][:],
                                Copy,
                                accum_out=acc["dot_0"][:, csl],
                            )
                        for j in (1, 2):
                            nc.scalar.activation(
                                dummy.broadcast_to(prods[j][:].shape),
                                prods[j][:],
                                Copy,
                                accum_out=acc[f"dot_{j}"][:, csl],
                            )

            def _emit_small():
                # Batched: all RC row-chunks in one [P, RC*EMB] tile per tensor.
                W = RC * EMB
                tiles = {}
                for k in ("mu", "logvar", "prior_mu", "prior_logvar"):
                    t = smallp.tile([P, W], fp32, tag=f"sm_{k}", name=f"sm_{k}")
                    src = small_in[k][:, :].rearrange("(c p) e -> p c e", p=P)
                    nc.sync.dma_start(t[:].rearrange("p (c e) -> p c e", c=RC), src)
                    tiles[k] = t

                mu_t, lv_t = tiles["mu"], tiles["logvar"]
                pmu_t, plv_t = tiles["prior_mu"], tiles["prior_logvar"]

                def red(dst, src_t):
                    nc.vector.tensor_reduce(
                        dst[:, :], src_t[:].rearrange("p (c e) -> p c e", c=RC),
                        axis=AX, op=add,
                    )

                red(sacc["s_lv"], lv_t)
                red(sacc["s_plv"], plv_t)
                for src, key in ((mu_t, "s_mu2"), (pmu_t, "s_pmu2")):
                    sq = smallp.tile([P, W], fp32, tag="sq", name="sq")
                    nc.vector.tensor_tensor(sq[:], src[:], src[:], op=mult)
                    red(sacc[key], sq)
                d_t = smallp.tile([P, W], fp32, tag="d_t", name="d_t")
                nc.vector.tensor_tensor(d_t[:], mu_t[:], pmu_t[:], op=subtract)
                sqd = smallp.tile([P, W], fp32, tag="sq", name="sqd")
                nc.vector.tensor_tensor(sqd[:], d_t[:], d_t[:], op=mult)
                red(sacc["s_d2"], sqd)
                sum_t = smallp.tile([P, W], fp32, tag="sum_t", name="sum_t")
                nc.vector.tensor_tensor(sum_t[:], lv_t[:], plv_t[:], op=add)

                # ACT accum_out must be one scalar per partition -> per-chunk ops
                for rc in range(RC):
                    sl = slice(rc, rc + 1)
                    esl = slice(rc * EMB, (rc + 1) * EMB)
                    nc.scalar.activation(
                        dummy.broadcast_to(lv_t[:, esl].shape), lv_t[:, esl], Exp,
                        accum_out=sacc["s_elv"][:, sl],
                    )
                    nc.scalar.activation(
                        dummy.broadcast_to(plv_t[:, esl].shape), plv_t[:, esl], Exp,
                        accum_out=sacc["s_eplv"][:, sl],
                    )
                    nc.scalar.activation(
                        dummy.broadcast_to(sum_t[:, esl].shape), sum_t[:, esl], Exp,
                        scale=0.5, accum_out=sacc["s_g"][:, sl],
                    )

            for _rep in range(n_repeat):
                # small phase emitted mid-stream: its loads and ~22 tiny ops
                # fill DVE/ACT slack under the DMA floor instead of extending
                # the drain tail.
                for rc in range(RC):
                    _emit_big_rc(rc)
                    if rc == 1:
                        _emit_small()

            # ---- write partials out ----
            for k in _OUT_BIG:
                nc.sync.dma_start(outs[k][:, :], acc[k][:])
            for k in _OUT_SMALL:
                nc.sync.dma_start(outs[k][:, :], sacc[k][:])

    nc.compile()
    return nc


def _get_nc():
    global _CACHED_NC
    if _CACHED_NC is None:
        _CACHED_NC = _build_nc()
    return _CACHED_NC


LAST_RESULTS = None


def _combine(results):
    """Combine per-core per-row partial sums into the six scalars (float64)."""
    tot_bce = np.zeros(3, dtype=np.float64)
    tot_kld1 = 0.0
    tot_kld2 = 0.0
    tot_w = 0.0
    for r in results:
        xs = r["xs"].astype(np.float64).reshape(P, RC, CT).sum(-1)
        for j in range(3):
            dot = r[f"dot_{j}"].astype(np.float64).reshape(P, RC, CT).sum(-1)
            es = r[f"es_{j}"].astype(np.float64).reshape(P, RC, CT).sum(-1)
            tot_bce[j] += (dot - np.log(es) * xs).sum()
        s_lv = r["s_lv"].astype(np.float64)
        s_mu2 = r["s_mu2"].astype(np.float64)
        s_elv = r["s_elv"].astype(np.float64)
        s_plv = r["s_plv"].astype(np.float64)
        s_pmu2 = r["s_pmu2"].astype(np.float64)
        s_eplv = r["s_eplv"].astype(np.float64)
        s_d2 = r["s_d2"].astype(np.float64)
        s_g = r["s_g"].astype(np.float64)
        tot_kld1 += (EMB + s_lv - s_mu2 - s_elv).sum()
        tot_kld2 += (EMB + s_plv - s_pmu2 - s_eplv).sum()
        tot_w += (s_d2 + s_elv + s_eplv - 2.0 * s_g).sum()

    BCE_merged = -tot_bce[0] / (B * N)  # recon_x
    BCE_rec = -tot_bce[1] / (B * N)  # logits_rec
    BCE_text = -tot_bce[2] / (B * N)  # logits_text
    BCE = (BCE_merged + BCE_text + BCE_rec) / 3.0
    KLD1 = -0.5 * tot_kld1 / (B * EMB)
    KLD2 = -0.5 * tot_kld2 / (B * EMB)
    W = tot_w / B
    l = BCE + 0.5 * (KLD1 + KLD2) + W
    return tuple(np.float32(v) for v in (l, BCE, W, BCE_rec, BCE_text, BCE_merged))


def kernel(recon_x, logits_rec, logits_text, x, mu, logvar, prior_mu, prior_logvar):
    from concourse.bass_utils import run_bass_kernel_spmd

    global LAST_RESULTS
    full = {
        "recon_x": recon_x,
        "logits_rec": logits_rec,
        "logits_text": logits_text,
        "x": x,
        "mu": mu,
        "logvar": logvar,
        "prior_mu": prior_mu,
        "prior_logvar": prior_logvar,
    }
    full = {k: np.ascontiguousarray(np.asarray(v, dtype=np.float32)) for k, v in full.items()}

    in_maps = [
        {k: v[i * ROWS : (i + 1) * ROWS] for k, v in full.items()} for i in range(NCORES)
    ]
    nc = _get_nc()
    LAST_RESULTS = run_bass_kernel_spmd(nc, in_maps, list(range(NCORES)))
    return _combine(LAST_RESULTS.results)
